# revision 5
# baseline (speedup 1.0000x reference)
"""Trainium2 Bass kernel for nn_BAC_15152644620305.

Per batch element (1 per NeuronCore, 8 cores):
  p_dense = relu(p @ W1 + b1); q_dense = relu(q @ W2 + b2)
  A = (p_dense @ q_dense.T) / sqrt(600)
  passage_aligned = softmax_rows(A) @ passage ; query_aligned = softmax_cols(A).T @ query
  6 factorization-machine heads on {concat, diff, mul} pairs -> [L, 3] x 2 outputs.

Implementation notes:
  - All heavy matmuls in bf16 (1 cyc/row on PE), fp32 PSUM accumulation.
  - Affinity computed in BOTH layouts (cheaper than transposing exp(A) on-chip);
    exp without max-subtraction (affinity values are in [0.1, 1.1]).
  - exp(A) stored as fp8e4m3 (softmax weights only -> negligible error, halves SBUF).
  - Softmax denominators ride along as an extra ones-column in the aligned matmuls'
    stationary operand, landing at an aligned output partition (96).
  - FM heads algebraically reduced: the x^2 @ V^2.T term needs only sum_k V_k^2;
    diff projections are linear combos of the qa/p projections; all per-head
    combination is done by one small stationary matmul per output chunk.
"""
import numpy as np

L_FULL = 2048
D = 600
U = 300
KFM = 5
N_CORES = 8
SCALE = float(1.0 / np.sqrt(np.float32(D)))

DCH = [(0, 128), (128, 128), (256, 128), (384, 128), (512, 88)]   # D chunks
UCH = [(0, 128), (128, 128), (256, 44)]                           # U chunks
ONES_COL = 608        # column in the 640-wide natural tile holding the ones
ONES_ROW = 96         # output partition where the denominator row lands
NATW = 640
COMB_K = [12, 12, 2, 2, 7, 1, 10, 5]   # K-sizes of the 8 combine pieces


def _emit(nc, L):
    import concourse.bass as bass
    import concourse.mybir as mybir
    import concourse.tile as tile
    from concourse.masks import make_identity
    from contextlib import ExitStack

    f32 = mybir.dt.float32
    bf16 = mybir.dt.bfloat16
    fp8 = mybir.dt.float8e4
    AF = mybir.ActivationFunctionType
    ds = bass.ds

    LT = L // 128               # l tiles
    NCW = min(512, L)           # moving-dim chunk width
    NCX = L // NCW              # chunks per L
    TG = 4 if LT % 4 == 0 else 1  # l-tiles per transpose psum batch

    x_d = nc.dram_tensor("x", [2, L, D], f32, kind="ExternalInput")
    w_d = [nc.dram_tensor("w1", [D, U], f32, kind="ExternalInput"),
           nc.dram_tensor("w2", [D, U], f32, kind="ExternalInput")]
    b_d = [nc.dram_tensor("b1", [U], f32, kind="ExternalInput"),
           nc.dram_tensor("b2", [U], f32, kind="ExternalInput")]
    stat_d = nc.dram_tensor("stat", [2, D, 36], f32, kind="ExternalInput")
    comb_d = nc.dram_tensor("comb", [8, 12, 3], f32, kind="ExternalInput")
    w0_d = nc.dram_tensor("w0col", [3, 2], f32, kind="ExternalInput")
    out_d = nc.dram_tensor("out", [2, 3, L], f32, kind="ExternalOutput")

    with tile.TileContext(nc) as tc, ExitStack() as ctx:
        const = ctx.enter_context(tc.tile_pool(name="const", bufs=1))
        big = ctx.enter_context(tc.tile_pool(name="big", bufs=1))
        epool = ctx.enter_context(tc.tile_pool(name="epool", bufs=LT))
        natp = ctx.enter_context(tc.tile_pool(name="natp", bufs=LT))
        nf32p = ctx.enter_context(tc.tile_pool(name="nf32p", bufs=3))
        stg = ctx.enter_context(tc.tile_pool(name="stg", bufs=2))
        fmt = ctx.enter_context(tc.tile_pool(name="fmt", bufs=6))
        sp = ctx.enter_context(tc.tile_pool(name="sp", bufs=2))
        rp = ctx.enter_context(tc.tile_pool(name="rp", bufs=2))
        ob = ctx.enter_context(tc.tile_pool(name="ob", bufs=2))
        ps = ctx.enter_context(tc.tile_pool(name="ps", bufs=8, space="PSUM"))

        def pst(p_cnt=128, w=NCW):
            return ps.tile([p_cnt, w], f32, tag="ps", name="pst")

        # ---------------- constants ----------------
        ident = const.tile([128, 128], f32, tag="ident")
        make_identity(nc, ident)
        ones = const.tile([128, 128], f32, tag="ones")
        nc.vector.memset(ones[:], 1.0)
        w0sb = const.tile([3, 2], f32, tag="w0sb")
        nc.sync.dma_start(w0sb[:], w0_d[:])

        cb = []
        for i, ksz in enumerate(COMB_K):
            cstg = stg.tile([12, 3], f32, tag="stg_c", name="cstg")
            nc.sync.dma_start(cstg[:], comb_d[i])
            t = const.tile([12, 3], bf16, tag=f"cb{i}", name=f"cb{i}")
            nc.vector.tensor_copy(t[:], cstg[:])
            cb.append(t)

        Wsb = [[], []]
        for t in range(2):
            for k, (doff, dcnt) in enumerate(DCH):
                wstg = stg.tile([128, U], f32, tag="stg_w", name="wstg")
                nc.sync.dma_start(wstg[:dcnt, :], w_d[t][ds(doff, dcnt), :])
                wt = const.tile([128, U], bf16, tag=f"W{t}_{k}", name=f"W{t}_{k}")
                nc.vector.tensor_copy(wt[:dcnt, :], wstg[:dcnt, :])
                Wsb[t].append(wt)

        stat = [[], []]
        for s in range(2):
            for k, (doff, dcnt) in enumerate(DCH):
                sstg = stg.tile([128, 36], f32, tag="stg_s", name="sstg")
                nc.sync.dma_start(sstg[:dcnt, :], stat_d[s, ds(doff, dcnt), :])
                st = const.tile([128, 36], bf16, tag=f"st{s}_{k}", name=f"st{s}_{k}")
                nc.vector.tensor_copy(st[:dcnt, :], sstg[:dcnt, :])
                stat[s].append(st)

        bsb = const.tile([128, 6], f32, tag="bsb")
        for t in range(2):
            for m, (uoff, ucnt) in enumerate(UCH):
                nc.sync.dma_start(bsb[:ucnt, t * 3 + m: t * 3 + m + 1],
                                  b_d[t][ds(uoff, ucnt)])

        # ---------------- phase 1: transpose inputs -> pT/qT (bf16 [d, L]) ----
        xT = [[], []]
        for t in range(2):
            for k in range(len(DCH)):
                xT[t].append(big.tile([128, L], bf16, tag=f"xT{t}_{k}",
                                      name=f"xT{t}_{k}"))
        for t in range(2):
            for g in range(LT // TG):
                pjs = [pst() for _ in range(len(DCH))]
                for ii in range(TG):
                    i = g * TG + ii
                    nf = nf32p.tile([128, D], f32, tag="nf", name="nf")
                    nc.sync.dma_start(nf[:], x_d[t, ds(i * 128, 128), :])
                    for k, (doff, dcnt) in enumerate(DCH):
                        nc.tensor.transpose(
                            pjs[k][:dcnt, ds(ii * 128, 128)],
                            nf[:, ds(doff, dcnt)], ident[:])
                for k, (doff, dcnt) in enumerate(DCH):
                    nc.scalar.copy(xT[t][k][:dcnt, ds(g * TG * 128, TG * 128)],
                                   pjs[k][:dcnt, ds(0, TG * 128)])

        # ---------------- phase 2: dense -> pdT/qdT (bf16 [u, L]) -------------
        dT = [[], []]
        for t in range(2):
            for m in range(len(UCH)):
                dT[t].append(big.tile([128, L], bf16, tag=f"dT{t}_{m}",
                                      name=f"dT{t}_{m}"))
        for t in range(2):
            for m, (uoff, ucnt) in enumerate(UCH):
                for nx in range(NCX):
                    acc = pst()
                    for k, (doff, dcnt) in enumerate(DCH):
                        nc.tensor.matmul(
                            acc[:ucnt, :],
                            Wsb[t][k][:dcnt, ds(uoff, ucnt)],
                            xT[t][k][:dcnt, ds(nx * NCW, NCW)],
                            start=(k == 0), stop=(k == len(DCH) - 1))
                    nc.scalar.activation(
                        dT[t][m][:ucnt, ds(nx * NCW, NCW)], acc[:ucnt, :],
                        AF.Relu, bias=bsb[:ucnt, t * 3 + m: t * 3 + m + 1])

        # helpers ------------------------------------------------------------
        def affinity_to_E(lhs_t, rhs_t, tagged):
            """E[i] tiles [128, L] fp8 = exp(SCALE * lhs.T @ rhs) per l-tile."""
            E = []
            for i in range(LT):
                e = epool.tile([128, L], fp8, tag="E", name=f"E{tagged}_{i}")
                for nx in range(NCX):
                    acc = pst()
                    for m, (uoff, ucnt) in enumerate(UCH):
                        nc.tensor.matmul(
                            acc[:, :],
                            lhs_t[m][:ucnt, ds(i * 128, 128)],
                            rhs_t[m][:ucnt, ds(nx * NCW, NCW)],
                            start=(m == 0), stop=(m == len(UCH) - 1))
                    nc.scalar.activation(e[:, ds(nx * NCW, NCW)], acc[:, :],
                                         AF.Exp, scale=SCALE)
                E.append(e)
            return E

        def build_nat(t):
            """natural-layout bf16 tiles [128, 640] with ones col, from x[t]."""
            nats = []
            for i in range(LT):
                nf = nf32p.tile([128, D], f32, tag="nf", name="nfn")
                nc.sync.dma_start(nf[:], x_d[t, ds(i * 128, 128), :])
                nt = natp.tile([128, NATW], bf16, tag="nat", name=f"nat{t}_{i}")
                nc.vector.memset(nt[:], 0.0)
                nc.vector.tensor_copy(nt[:, 0:D], nf[:])
                nc.vector.memset(nt[:, ONES_COL:ONES_COL + 1], 1.0)
                nats.append(nt)
            return nats

        def aligned_T(nats, E, side_tag):
            """alT tiles [d,L] bf16 = normalized aligned.T, via ones-row trick."""
            alT = [big.tile([128, L], bf16, tag=f"alT{k}", name=f"alT{side_tag}{k}")
                   for k in range(len(DCH))]
            R = big.tile([128, L], bf16, tag="R", name=f"R{side_tag}")
            # pass A: last d-chunk (88 rows) + ones row at partition 96
            ps4 = [pst() for _ in range(NCX)]
            for i in range(LT):
                for nx in range(NCX):
                    nc.tensor.matmul(ps4[nx][:, :],
                                     nats[i][:, ds(512, 128)],
                                     E[i][:, ds(nx * NCW, NCW)],
                                     start=(i == 0), stop=(i == LT - 1))
            for nx in range(NCX):
                rr = rp.tile([128, NCW], f32, tag="rr", name="rr")
                nc.vector.reciprocal(rr[ONES_ROW:ONES_ROW + 1, :],
                                     ps4[nx][ONES_ROW:ONES_ROW + 1, :])
                bc = pst()
                nc.tensor.matmul(bc[:, :], ones[ONES_ROW:ONES_ROW + 1, 0:128],
                                 rr[ONES_ROW:ONES_ROW + 1, :],
                                 start=True, stop=True,
                                 tile_position=(ONES_ROW, 0))
                nc.scalar.copy(R[:, ds(nx * NCW, NCW)], bc[:, :])
                nc.vector.tensor_mul(alT[4][0:88, ds(nx * NCW, NCW)],
                                     ps4[nx][0:88, :], R[0:88, ds(nx * NCW, NCW)])
            # passes B, C: d-chunks 0..3, two at a time
            for mm0 in (0, 2):
                accs = {}
                for m in (mm0, mm0 + 1):
                    for nx in range(NCX):
                        accs[(m, nx)] = pst()
                for i in range(LT):
                    for m in (mm0, mm0 + 1):
                        for nx in range(NCX):
                            nc.tensor.matmul(accs[(m, nx)][:, :],
                                             nats[i][:, ds(m * 128, 128)],
                                             E[i][:, ds(nx * NCW, NCW)],
                                             start=(i == 0), stop=(i == LT - 1))
                for m in (mm0, mm0 + 1):
                    for nx in range(NCX):
                        nc.vector.tensor_mul(alT[m][:, ds(nx * NCW, NCW)],
                                             accs[(m, nx)][:, :],
                                             R[:, ds(nx * NCW, NCW)])
            return alT

        def fm_side(s, xTs, bTs):
            """FM heads for one side: x = aligned.T tiles, b = own input.T."""
            for nx in range(NCX):
                nsl = ds(nx * NCW, NCW)
                gX = ps.tile([12, NCW], f32, tag="ps", name="gX")
                gB = ps.tile([12, NCW], f32, tag="ps", name="gB")
                gX2 = ps.tile([2, NCW], f32, tag="ps", name="gX2")
                gB2 = ps.tile([2, NCW], f32, tag="ps", name="gB2")
                gM = ps.tile([7, NCW], f32, tag="ps", name="gM")
                gM2 = ps.tile([1, NCW], f32, tag="ps", name="gM2")
                nk = len(DCH)
                for k, (doff, dcnt) in enumerate(DCH):
                    x_sl = xTs[k][:dcnt, nsl]
                    b_sl = bTs[k][:dcnt, nsl]
                    tx2 = fmt.tile([128, NCW], bf16, tag="fmt", name="tx2")
                    tb2 = fmt.tile([128, NCW], bf16, tag="fmt", name="tb2")
                    txm = fmt.tile([128, NCW], bf16, tag="fmt", name="txm")
                    txm2 = fmt.tile([128, NCW], bf16, tag="fmt", name="txm2")
                    nc.vector.tensor_mul(tx2[:dcnt, :], x_sl, x_sl)
                    nc.vector.tensor_mul(tb2[:dcnt, :], b_sl, b_sl)
                    nc.vector.tensor_mul(txm[:dcnt, :], x_sl, b_sl)
                    nc.vector.tensor_mul(txm2[:dcnt, :], txm[:dcnt, :],
                                         txm[:dcnt, :])
                    st = stat[s][k]
                    fl = (k == 0, k == nk - 1)
                    nc.tensor.matmul(gX[:, :], st[:dcnt, 0:12], x_sl,
                                     start=fl[0], stop=fl[1])
                    nc.tensor.matmul(gB[:, :], st[:dcnt, 12:24], b_sl,
                                     start=fl[0], stop=fl[1])
                    nc.tensor.matmul(gX2[:, :], st[:dcnt, 24:26], tx2[:dcnt, :],
                                     start=fl[0], stop=fl[1])
                    nc.tensor.matmul(gB2[:, :], st[:dcnt, 26:28], tb2[:dcnt, :],
                                     start=fl[0], stop=fl[1])
                    nc.tensor.matmul(gM[:, :], st[:dcnt, 28:35], txm[:dcnt, :],
                                     start=fl[0], stop=fl[1])
                    nc.tensor.matmul(gM2[:, :], st[:dcnt, 35:36], txm2[:dcnt, :],
                                     start=fl[0], stop=fl[1])
                sX = sp.tile([12, NCW], bf16, tag="sX", name="sX")
                sB = sp.tile([12, NCW], bf16, tag="sB", name="sB")
                sX2 = sp.tile([2, NCW], bf16, tag="sX2", name="sX2")
                sB2 = sp.tile([2, NCW], bf16, tag="sB2", name="sB2")
                sM = sp.tile([7, NCW], bf16, tag="sM", name="sM")
                sM2 = sp.tile([1, NCW], bf16, tag="sM2", name="sM2")
                nc.scalar.copy(sX[:, :], gX[:, :])
                nc.scalar.copy(sB[:, :], gB[:, :])
                nc.scalar.copy(sX2[:, :], gX2[:, :])
                nc.scalar.copy(sB2[:, :], gB2[:, :])
                nc.scalar.copy(sM[:, :], gM[:, :])
                nc.scalar.copy(sM2[:, :], gM2[:, :])
                # B-group Vd columns carry -Vd, so diff quads are also an add
                TA = sp.tile([10, NCW], f32, tag="TA", name="TA")
                nc.vector.tensor_add(TA[0:10, :], sX[0:10, :], sB[0:10, :])
                TQ = sp.tile([10, NCW], bf16, tag="TQ", name="TQ")
                nc.scalar.activation(TQ[:, :], TA[:, :], AF.Square)
                TQM = sp.tile([5, NCW], bf16, tag="TQM", name="TQM")
                nc.scalar.activation(TQM[:, :], sM[0:5, :], AF.Square)
                cps = ps.tile([3, NCW], f32, tag="ps", name="cps")
                pieces = [(cb[0], sX, 12), (cb[1], sB, 12), (cb[2], sX2, 2),
                          (cb[3], sB2, 2), (cb[4], sM, 7), (cb[5], sM2, 1),
                          (cb[6], TQ, 10), (cb[7], TQM, 5)]
                for pi, (cpc, rhs_t, ksz) in enumerate(pieces):
                    nc.tensor.matmul(cps[:, :], cpc[0:ksz, :], rhs_t[0:ksz, :],
                                     start=(pi == 0), stop=(pi == len(pieces) - 1))
                o = ob.tile([3, NCW], f32, tag="ob", name="o")
                nc.scalar.activation(o[:, :], cps[:, :], AF.Identity,
                                     bias=w0sb[:, s:s + 1])
                nc.sync.dma_start(out_d[s, :, nsl], o[:, :])

        # ---------------- main flow ----------------
        E1 = affinity_to_E(dT[0], dT[1], "1")     # E1[p-tile][p, q]
        q_nats = build_nat(1)
        qaT = aligned_T(q_nats, E1, "q")          # query_aligned.T
        fm_side(0, qaT, xT[0])                    # passage-side features
        E2 = affinity_to_E(dT[1], dT[0], "2")     # E2[q-tile][q, p]
        p_nats = build_nat(0)
        paT = aligned_T(p_nats, E2, "p")          # passage_aligned.T
        fm_side(1, paT, xT[1])                    # query-side features


def _host_prep(W1, b1, W2, b2, cat_w0, cat_w, cat_V, dm_w0, dm_w, dm_V):
    stat = np.zeros((2, D, 36), np.float32)
    for s in range(2):
        ci, di, mi = s, s, s + 2
        Va = cat_V[ci][:, :D]
        Vb = cat_V[ci][:, D:]
        Vd = dm_V[di]
        Vm = dm_V[mi]
        stat[s, :, 0:5] = Va.T
        stat[s, :, 5:10] = Vd.T
        stat[s, :, 10] = cat_w[ci, :D]
        stat[s, :, 11] = dm_w[di]
        stat[s, :, 12:17] = Vb.T
        stat[s, :, 17:22] = -Vd.T   # negated: quad build is then a single add
        stat[s, :, 22] = cat_w[ci, D:]
        stat[s, :, 23] = dm_w[di]
        stat[s, :, 24] = (Va ** 2).sum(0)
        stat[s, :, 25] = (Vd ** 2).sum(0)
        stat[s, :, 26] = (Vb ** 2).sum(0)
        stat[s, :, 27] = (Vd ** 2).sum(0)
        stat[s, :, 28:33] = Vm.T
        stat[s, :, 33] = dm_w[mi]
        stat[s, :, 34] = (Vd ** 2).sum(0)
        stat[s, :, 35] = (Vm ** 2).sum(0)

    comb = np.zeros((8, 12, 3), np.float32)
    comb[0, 10, 0] = 1.0    # CX: x@w_cat -> c0
    comb[0, 11, 1] = 1.0    #     x@w_d   -> c1
    comb[1, 10, 0] = 1.0    # CB
    comb[1, 11, 1] = -1.0
    comb[2, 0, 0] = -0.5    # CX2: x2@u_cat
    comb[2, 1, 1] = -0.5    #      x2@u_d
    comb[3, 0, 0] = -0.5    # CB2
    comb[3, 1, 1] = -0.5
    comb[4, 5, 2] = 1.0     # CM: mul@w_m -> c2
    comb[4, 6, 1] = 1.0     #     mul@u_d -> c1
    comb[5, 0, 2] = -0.5    # CM2: mul2@u_m
    comb[6, 0:5, 0] = 0.5   # CQ: cat quads
    comb[6, 5:10, 1] = 0.5  #     diff quads
    comb[7, 0:5, 2] = 0.5   # CQM: mul quads

    w0col = np.zeros((3, 2), np.float32)
    for s in range(2):
        w0col[0, s] = cat_w0[s, 0]
        w0col[1, s] = dm_w0[s, 0]
        w0col[2, s] = dm_w0[s + 2, 0]
    return stat, comb, w0col


_PROG = None


def _get_prog():
    global _PROG
    if _PROG is None:
        from concourse import bacc
        nc = bacc.Bacc(None, target_bir_lowering=False)
        _emit(nc, L_FULL)
        nc.finalize()
        _PROG = nc
    return _PROG


def _in_maps(stack_input, W1, b1, W2, b2, fm_cat_w0, fm_cat_w, fm_cat_V,
             fm_dm_w0, fm_dm_w, fm_dm_V):
    f = lambda a: np.ascontiguousarray(np.asarray(a, np.float32))
    stack_input = f(stack_input)
    stat, comb, w0col = _host_prep(f(W1), f(b1), f(W2), f(b2), f(fm_cat_w0),
                                   f(fm_cat_w), f(fm_cat_V), f(fm_dm_w0),
                                   f(fm_dm_w), f(fm_dm_V))
    common = {"w1": f(W1), "w2": f(W2), "b1": f(b1), "b2": f(b2),
              "stat": stat, "comb": comb, "w0col": w0col}
    return [dict(common, x=np.ascontiguousarray(stack_input[:, b]))
            for b in range(N_CORES)]


def kernel(stack_input, W1, b1, W2, b2, fm_cat_w0, fm_cat_w, fm_cat_V,
           fm_dm_w0, fm_dm_w, fm_dm_V):
    from concourse.bass_utils import run_bass_kernel_spmd

    in_maps = _in_maps(stack_input, W1, b1, W2, b2, fm_cat_w0, fm_cat_w,
                       fm_cat_V, fm_dm_w0, fm_dm_w, fm_dm_V)
    nc = _get_prog()
    res = run_bass_kernel_spmd(nc, in_maps, core_ids=list(range(N_CORES)))
    outs = [r["out"] for r in res.results]            # each [2, 3, L]
    fp = np.stack([o[0].T for o in outs]).astype(np.float32)   # [8, L, 3]
    fq = np.stack([o[1].T for o in outs]).astype(np.float32)
    return fp, fq


# revision 21
# speedup vs baseline: 1.0444x; 1.0444x over previous
"""Trainium2 Bass kernel for nn_BAC_15152644620305.

Per batch element (1 per NeuronCore, 8 cores):
  p_dense = relu(p @ W1 + b1); q_dense = relu(q @ W2 + b2)
  A = (p_dense @ q_dense.T) / sqrt(600)
  passage_aligned = softmax_rows(A) @ passage ; query_aligned = softmax_cols(A).T @ query
  6 factorization-machine heads on {concat, diff, mul} pairs -> [L, 3] x 2 outputs.

Implementation notes:
  - All heavy matmuls in bf16 (1 cyc/row on PE), fp32 PSUM accumulation.
  - Affinity computed in BOTH layouts (cheaper than transposing exp(A) on-chip);
    exp without max-subtraction (affinity values are in [0.1, 1.1]).
  - exp(A) stored as fp8e4m3 (softmax weights only -> negligible error, halves SBUF).
  - Softmax denominators ride along as an extra ones-column in the aligned matmuls'
    stationary operand, landing at an aligned output partition (96).
  - FM heads algebraically reduced: the x^2 @ V^2.T term needs only sum_k V_k^2;
    diff projections are linear combos of the qa/p projections; all per-head
    combination is done by one small stationary matmul per output chunk.
"""
import numpy as np

L_FULL = 2048
D = 600
U = 300
KFM = 5
N_CORES = 8
SCALE = float(1.0 / np.sqrt(np.float32(D)))

DCH = [(0, 128), (128, 128), (256, 128), (384, 128), (512, 88)]   # D chunks
UCH = [(0, 128), (128, 128), (256, 44)]                           # U chunks
ONES_COL = 608        # column in the 640-wide natural tile holding the ones
ONES_ROW = 96         # output partition where the denominator row lands
NATW = 640


def _emit(nc, L):
    import concourse.bass as bass
    import concourse.mybir as mybir
    import concourse.tile as tile
    from concourse.masks import make_identity
    from contextlib import ExitStack

    f32 = mybir.dt.float32
    bf16 = mybir.dt.bfloat16
    fp8 = mybir.dt.float8e4
    AF = mybir.ActivationFunctionType
    ds = bass.ds

    LT = L // 128               # l tiles
    NCW = min(512, L)           # moving-dim chunk width
    NCX = L // NCW              # chunks per L
    TG = 4 if LT % 4 == 0 else 1  # l-tiles per transpose psum batch

    x_d = nc.dram_tensor("x", [2, L, D], f32, kind="ExternalInput")
    wp_d = nc.dram_tensor("wpack", [10, 128, U], f32, kind="ExternalInput")
    sp_d = nc.dram_tensor("statp", [10, 128, 36], f32, kind="ExternalInput")
    c2_d = nc.dram_tensor("comb2", [128, 6], f32, kind="ExternalInput")
    bp_d = nc.dram_tensor("biasp", [128, 6], f32, kind="ExternalInput")
    w0_d = nc.dram_tensor("w0col", [3, 2], f32, kind="ExternalInput")
    out_d = nc.dram_tensor("out", [2, 3, L], f32, kind="ExternalOutput")

    with tile.TileContext(nc) as tc, ExitStack() as ctx:
        const = ctx.enter_context(tc.tile_pool(name="const", bufs=1))
        big = ctx.enter_context(tc.tile_pool(name="big", bufs=1))
        epool = ctx.enter_context(tc.tile_pool(name="epool", bufs=LT))
        natp = ctx.enter_context(tc.tile_pool(name="natp", bufs=LT))
        nf32p = ctx.enter_context(tc.tile_pool(name="nf32p", bufs=3))
        stg = ctx.enter_context(tc.tile_pool(name="stg", bufs=2))
        fmt = ctx.enter_context(tc.tile_pool(name="fmt", bufs=6))
        sp = ctx.enter_context(tc.tile_pool(name="sp", bufs=2))
        rp = ctx.enter_context(tc.tile_pool(name="rp", bufs=2))
        ob = ctx.enter_context(tc.tile_pool(name="ob", bufs=2))
        ps = ctx.enter_context(tc.tile_pool(name="ps", bufs=8, space="PSUM"))

        def pst(p_cnt=128, w=NCW):
            return ps.tile([p_cnt, w], f32, tag="ps", name="pst")

        # ------- constants (packed loads on the scalar HWDGE queue) -------
        identb = const.tile([128, 128], bf16, tag="identb")
        make_identity(nc, identb)
        onesb = const.tile([128, 128], bf16, tag="onesb")
        nc.vector.memset(onesb[:], 1.0)
        w0sb = const.tile([3, 2], f32, tag="w0sb")
        nc.scalar.dma_start(w0sb[:], w0_d[:])

        wstg = stg.tile([128, 10 * U], f32, tag="stg_w", name="wstg", bufs=1)
        nc.scalar.dma_start(
            wstg[:].rearrange("p (t c) -> p t c", t=10),
            wp_d[:].rearrange("t p c -> p t c"))
        Wall = const.tile([128, 10 * U], bf16, tag="Wall")
        nc.vector.tensor_copy(Wall[:], wstg[:])
        Wsb = [[Wall[:, ds((t * 5 + k) * U, U)] for k in range(5)]
               for t in range(2)]

        sstg = stg.tile([128, 360], f32, tag="stg_s", name="sstg", bufs=1)
        nc.scalar.dma_start(
            sstg[:].rearrange("p (t c) -> p t c", t=10),
            sp_d[:].rearrange("t p c -> p t c"))
        Sall = const.tile([128, 360], bf16, tag="Sall")
        nc.vector.tensor_copy(Sall[:], sstg[:])
        stat = [[Sall[:, ds((s * 5 + k) * 36, 36)] for k in range(5)]
                for s in range(2)]

        cstg = stg.tile([128, 6], f32, tag="stg_c", name="cstg", bufs=1)
        nc.scalar.dma_start(cstg[:], c2_d[:])
        cb2 = const.tile([128, 6], bf16, tag="cb2")
        nc.vector.tensor_copy(cb2[:], cstg[:])

        bsb = const.tile([128, 6], f32, tag="bsb")
        nc.scalar.dma_start(bsb[:], bp_d[:])

        # ---------------- phase 1: transpose inputs -> pT/qT (bf16 [d, L]) ----
        xT = [[], []]
        for t in range(2):
            for k in range(len(DCH)):
                xT[t].append(big.tile([128, L], bf16, tag=f"xT{t}_{k}",
                                      name=f"xT{t}_{k}"))
        # phase 1+2 interleaved per l-group: transpose inputs -> pT/qT, then
        # the dense matmuls for that group's columns (keeps PE fed during the
        # next group's DMA + cast)
        dT = [[], []]
        for t in range(2):
            for m in range(len(UCH)):
                dT[t].append(big.tile([128, L], bf16, tag=f"dT{t}_{m}",
                                      name=f"dT{t}_{m}"))
        for g in range(LT // TG):
            gw = TG * 128
            for t in range(2):
                pjs = [ps.tile([128, NCW], bf16, tag="ps", name="pjs")
                       for _ in range(len(DCH))]
                for ii in range(TG):
                    i = g * TG + ii
                    nf = nf32p.tile([128, D], f32, tag="nf", name="nf")
                    nc.sync.dma_start(nf[:], x_d[t, ds(i * 128, 128), :])
                    nfb = nf32p.tile([128, D], bf16, tag="nfb", name="nfb")
                    nc.vector.tensor_copy(nfb[:], nf[:])
                    for k, (doff, dcnt) in enumerate(DCH):
                        nc.tensor.transpose(
                            pjs[k][:dcnt, ds(ii * 128, 128)],
                            nfb[:, ds(doff, dcnt)], identb[:])
                for k, (doff, dcnt) in enumerate(DCH):
                    nc.scalar.copy(xT[t][k][:dcnt, ds(g * gw, gw)],
                                   pjs[k][:dcnt, ds(0, gw)])
            if gw == NCW:
                for t in range(2):
                    for m, (uoff, ucnt) in enumerate(UCH):
                        acc = pst()
                        for k, (doff, dcnt) in enumerate(DCH):
                            nc.tensor.matmul(
                                acc[:ucnt, :],
                                Wsb[t][k][:dcnt, ds(uoff, ucnt)],
                                xT[t][k][:dcnt, ds(g * NCW, NCW)],
                                start=(k == 0), stop=(k == len(DCH) - 1))
                        nc.scalar.activation(
                            dT[t][m][:ucnt, ds(g * NCW, NCW)], acc[:ucnt, :],
                            AF.Relu, bias=bsb[:ucnt, t * 3 + m: t * 3 + m + 1])
        if TG * 128 != NCW:
            for t in range(2):
                for m, (uoff, ucnt) in enumerate(UCH):
                    for nx in range(NCX):
                        acc = pst()
                        for k, (doff, dcnt) in enumerate(DCH):
                            nc.tensor.matmul(
                                acc[:ucnt, :],
                                Wsb[t][k][:dcnt, ds(uoff, ucnt)],
                                xT[t][k][:dcnt, ds(nx * NCW, NCW)],
                                start=(k == 0), stop=(k == len(DCH) - 1))
                        nc.scalar.activation(
                            dT[t][m][:ucnt, ds(nx * NCW, NCW)], acc[:ucnt, :],
                            AF.Relu, bias=bsb[:ucnt, t * 3 + m: t * 3 + m + 1])

        # helpers ------------------------------------------------------------
        def affinity_to_E(lhs_t, rhs_t, tagged):
            """E[i] tiles [128, L] fp8 = exp(SCALE * lhs.T @ rhs) per l-tile."""
            E = []
            for i in range(LT):
                e = epool.tile([128, L], fp8, tag="E", name=f"E{tagged}_{i}")
                for nx in range(NCX):
                    acc = pst()
                    for m, (uoff, ucnt) in enumerate(UCH):
                        nc.tensor.matmul(
                            acc[:, :],
                            lhs_t[m][:ucnt, ds(i * 128, 128)],
                            rhs_t[m][:ucnt, ds(nx * NCW, NCW)],
                            start=(m == 0), stop=(m == len(UCH) - 1))
                    nc.scalar.activation(e[:, ds(nx * NCW, NCW)], acc[:, :],
                                         AF.Exp, scale=SCALE)
                E.append(e)
            return E

        def build_nat(t):
            """natural-layout bf16 tiles [128, 640] with ones col, from x[t]."""
            nats = []
            for i in range(LT):
                nf = nf32p.tile([128, D], f32, tag="nf", name="nfn")
                nc.sync.dma_start(nf[:], x_d[t, ds(i * 128, 128), :])
                nt = natp.tile([128, NATW], bf16, tag="nat", name=f"nat{t}_{i}")
                nc.vector.memset(nt[:], 0.0)
                nc.vector.tensor_copy(nt[:, 0:D], nf[:])
                nc.vector.memset(nt[:, ONES_COL:ONES_COL + 1], 1.0)
                nats.append(nt)
            return nats

        def aligned_T(nats, E, side_tag):
            """alT tiles [d,L] bf16 = normalized aligned.T, via ones-row trick."""
            alT = [big.tile([128, L], bf16, tag=f"alT{k}", name=f"alT{side_tag}{k}")
                   for k in range(len(DCH))]
            R = big.tile([128, L], bf16, tag="R", name=f"R{side_tag}")
            # pass A: last d-chunk (88 rows) + ones row at partition 96
            ps4 = [pst() for _ in range(NCX)]
            for i in range(LT):
                for nx in range(NCX):
                    nc.tensor.matmul(ps4[nx][:, :],
                                     nats[i][:, ds(512, 128)],
                                     E[i][:, ds(nx * NCW, NCW)],
                                     start=(i == 0), stop=(i == LT - 1))
            for nx in range(NCX):
                rr = rp.tile([128, NCW], f32, tag="rr", name="rr")
                nc.vector.reciprocal(rr[ONES_ROW:ONES_ROW + 1, :],
                                     ps4[nx][ONES_ROW:ONES_ROW + 1, :])
                rrb = rp.tile([128, NCW], bf16, tag="rrb", name="rrb")
                nc.scalar.copy(rrb[ONES_ROW:ONES_ROW + 1, :],
                               rr[ONES_ROW:ONES_ROW + 1, :])
                bc = pst()
                nc.tensor.matmul(bc[:, :], onesb[ONES_ROW:ONES_ROW + 1, 0:128],
                                 rrb[ONES_ROW:ONES_ROW + 1, :],
                                 start=True, stop=True,
                                 tile_position=(ONES_ROW, 0))
                nc.scalar.copy(R[:, ds(nx * NCW, NCW)], bc[:, :])
                nc.vector.tensor_mul(alT[4][0:88, ds(nx * NCW, NCW)],
                                     ps4[nx][0:88, :], R[0:88, ds(nx * NCW, NCW)])
            # passes B, C: d-chunks 0..3, two at a time
            for mm0 in (0, 2):
                accs = {}
                for m in (mm0, mm0 + 1):
                    for nx in range(NCX):
                        accs[(m, nx)] = pst()
                for i in range(LT):
                    for m in (mm0, mm0 + 1):
                        for nx in range(NCX):
                            nc.tensor.matmul(accs[(m, nx)][:, :],
                                             nats[i][:, ds(m * 128, 128)],
                                             E[i][:, ds(nx * NCW, NCW)],
                                             start=(i == 0), stop=(i == LT - 1))
                for m in (mm0, mm0 + 1):
                    for nx in range(NCX):
                        nc.vector.tensor_mul(alT[m][:, ds(nx * NCW, NCW)],
                                             accs[(m, nx)][:, :],
                                             R[:, ds(nx * NCW, NCW)])
            return alT

        def fm_side(s, xTs, bTs):
            """FM heads for one side: x = aligned.T tiles, b = own input.T."""
            for nx in range(NCX):
                nsl = ds(nx * NCW, NCW)
                # col-packed projection groups: two psum tiles, three
                # concurrent col-groups each (col 96 / quadrant 3 avoided)
                # P1: X@0 (12), B@32 (12), M2@64 (1)
                # P2: X2@0 (2), B2@32 (2), M@64 (7)
                P1 = ps.tile([128, NCW], f32, tag="ps", name="P1")
                P2 = ps.tile([128, NCW], f32, tag="ps", name="P2")
                nk = len(DCH)
                for k, (doff, dcnt) in enumerate(DCH):
                    x_sl = xTs[k][:dcnt, nsl]
                    b_sl = bTs[k][:dcnt, nsl]
                    tx2 = fmt.tile([128, NCW], bf16, tag="fmt", name="tx2")
                    tb2 = fmt.tile([128, NCW], bf16, tag="fmt", name="tb2")
                    txm = fmt.tile([128, NCW], bf16, tag="fmt", name="txm")
                    txm2 = fmt.tile([128, NCW], bf16, tag="fmt", name="txm2")
                    nc.vector.tensor_mul(tx2[:dcnt, :], x_sl, x_sl)
                    nc.vector.tensor_mul(tb2[:dcnt, :], b_sl, b_sl)
                    nc.vector.tensor_mul(txm[:dcnt, :], x_sl, b_sl)
                    nc.vector.tensor_mul(txm2[:dcnt, :], txm[:dcnt, :],
                                         txm[:dcnt, :])
                    st = stat[s][k]
                    fl = (k == 0, k == nk - 1)
                    nc.tensor.matmul(P1[0:12, :], st[:dcnt, 0:12], x_sl,
                                     start=fl[0], stop=fl[1],
                                     tile_position=(0, 0),
                                     skip_group_check=True)
                    nc.tensor.matmul(P1[32:44, :], st[:dcnt, 12:24], b_sl,
                                     start=fl[0], stop=fl[1],
                                     tile_position=(0, 32),
                                     skip_group_check=True)
                    nc.tensor.matmul(P1[64:65, :], st[:dcnt, 35:36],
                                     txm2[:dcnt, :], start=fl[0], stop=fl[1],
                                     tile_position=(0, 64),
                                     skip_group_check=True)
                    nc.tensor.matmul(P2[0:2, :], st[:dcnt, 24:26],
                                     tx2[:dcnt, :], start=fl[0], stop=fl[1],
                                     tile_position=(0, 0),
                                     skip_group_check=True)
                    nc.tensor.matmul(P2[32:34, :], st[:dcnt, 26:28],
                                     tb2[:dcnt, :], start=fl[0], stop=fl[1],
                                     tile_position=(0, 32),
                                     skip_group_check=True)
                    nc.tensor.matmul(P2[64:71, :], st[:dcnt, 28:35],
                                     txm[:dcnt, :], start=fl[0], stop=fl[1],
                                     tile_position=(0, 64),
                                     skip_group_check=True)
                # pack group evictions at 32-aligned partition offsets so the
                # whole combine is 2 matmuls: S1 = [X@0, B@32, X2@64, B2@96],
                # S2 = [M@0, M2@32, TQ@64, TQM@96]
                S1 = sp.tile([128, NCW], bf16, tag="S1", name="S1")
                S2 = sp.tile([128, NCW], bf16, tag="S2", name="S2")
                nc.vector.memset(S1[:], 0.0)
                nc.vector.memset(S2[:], 0.0)
                nc.scalar.copy(S1[0:12, :], P1[0:12, :])
                nc.scalar.copy(S1[32:44, :], P1[32:44, :])
                nc.scalar.copy(S1[64:66, :], P2[0:2, :])
                nc.scalar.copy(S1[96:98, :], P2[32:34, :])
                nc.scalar.copy(S2[0:7, :], P2[64:71, :])
                nc.scalar.copy(S2[32:33, :], P1[64:65, :])
                # B-group Vd columns carry -Vd, so diff quads are also an add.
                # in0 from PSUM: two SBUF inputs must share a base partition.
                TA = sp.tile([10, NCW], f32, tag="TA", name="TA")
                nc.vector.tensor_add(TA[0:10, :], P1[0:10, :], S1[32:42, :])
                nc.scalar.activation(S2[64:74, :], TA[:, :], AF.Square)
                nc.scalar.activation(S2[96:101, :], S2[0:5, :], AF.Square)
                cps = ps.tile([3, NCW], f32, tag="ps", name="cps")
                nc.tensor.matmul(cps[:, :], cb2[0:98, 0:3], S1[0:98, :],
                                 start=True, stop=False)
                nc.tensor.matmul(cps[:, :], cb2[0:101, 3:6], S2[0:101, :],
                                 start=False, stop=True)
                o = ob.tile([3, NCW], f32, tag="ob", name="o")
                nc.scalar.activation(o[:, :], cps[:, :], AF.Identity,
                                     bias=w0sb[:, s:s + 1])
                nc.sync.dma_start(out_d[s, :, nsl], o[:, :])

        # ---------------- main flow ----------------
        E1 = affinity_to_E(dT[0], dT[1], "1")     # E1[p-tile][p, q]
        q_nats = build_nat(1)
        qaT = aligned_T(q_nats, E1, "q")          # query_aligned.T
        fm_side(0, qaT, xT[0])                    # passage-side features
        E2 = affinity_to_E(dT[1], dT[0], "2")     # E2[q-tile][q, p]
        p_nats = build_nat(0)
        paT = aligned_T(p_nats, E2, "p")          # passage_aligned.T
        fm_side(1, paT, xT[1])                    # query-side features


def _host_prep(W1, b1, W2, b2, cat_w0, cat_w, cat_V, dm_w0, dm_w, dm_V):
    stat = np.zeros((2, D, 36), np.float32)
    for s in range(2):
        ci, di, mi = s, s, s + 2
        Va = cat_V[ci][:, :D]
        Vb = cat_V[ci][:, D:]
        Vd = dm_V[di]
        Vm = dm_V[mi]
        stat[s, :, 0:5] = Va.T
        stat[s, :, 5:10] = Vd.T
        stat[s, :, 10] = cat_w[ci, :D]
        stat[s, :, 11] = dm_w[di]
        stat[s, :, 12:17] = Vb.T
        stat[s, :, 17:22] = -Vd.T   # negated: quad build is then a single add
        stat[s, :, 22] = cat_w[ci, D:]
        stat[s, :, 23] = dm_w[di]
        stat[s, :, 24] = (Va ** 2).sum(0)
        stat[s, :, 25] = (Vd ** 2).sum(0)
        stat[s, :, 26] = (Vb ** 2).sum(0)
        stat[s, :, 27] = (Vd ** 2).sum(0)
        stat[s, :, 28:33] = Vm.T
        stat[s, :, 33] = dm_w[mi]
        stat[s, :, 34] = (Vd ** 2).sum(0)
        stat[s, :, 35] = (Vm ** 2).sum(0)

    # packed combine matrices: S1 = [X@0, B@32, X2@64, B2@96],
    # S2 = [M@0, M2@32, TQ@64, TQM@96]
    comb2 = np.zeros((128, 6), np.float32)
    C1, C2 = comb2[:, 0:3], comb2[:, 3:6]
    C1[10, 0] = 1.0     # x@w_cat -> c_cat
    C1[11, 1] = 1.0     # x@w_d -> c_diff
    C1[32 + 10, 0] = 1.0
    C1[32 + 11, 1] = -1.0
    C1[64, 0] = -0.5    # x2@u_cat
    C1[65, 1] = -0.5    # x2@u_d
    C1[96, 0] = -0.5    # b2@u_cat
    C1[97, 1] = -0.5    # b2@u_d
    C2[5, 2] = 1.0      # mul@w_m
    C2[6, 1] = 1.0      # mul@u_d (from -0.5 * -2)
    C2[32, 2] = -0.5    # mul2@u_m
    C2[64:69, 0] = 0.5  # cat quads
    C2[69:74, 1] = 0.5  # diff quads
    C2[96:101, 2] = 0.5  # mul quads

    # packed per-d-chunk weights / stationaries / bias
    wpack = np.zeros((10, 128, U), np.float32)
    statp = np.zeros((10, 128, 36), np.float32)
    for t, W in enumerate((W1, W2)):
        for k, (doff, dcnt) in enumerate(DCH):
            wpack[t * 5 + k, :dcnt] = W[doff:doff + dcnt]
    for s in range(2):
        for k, (doff, dcnt) in enumerate(DCH):
            statp[s * 5 + k, :dcnt] = stat[s, doff:doff + dcnt]

    biasp = np.zeros((128, 6), np.float32)
    for t, b in enumerate((b1, b2)):
        for m, (uoff, ucnt) in enumerate(UCH):
            biasp[:ucnt, t * 3 + m] = b[uoff:uoff + ucnt]

    w0col = np.zeros((3, 2), np.float32)
    for s in range(2):
        w0col[0, s] = cat_w0[s, 0]
        w0col[1, s] = dm_w0[s, 0]
        w0col[2, s] = dm_w0[s + 2, 0]
    return wpack, statp, comb2, biasp, w0col


_PROG = None


def _get_prog():
    global _PROG
    if _PROG is None:
        from concourse import bacc
        nc = bacc.Bacc(None, target_bir_lowering=False)
        _emit(nc, L_FULL)
        nc.finalize()
        _PROG = nc
    return _PROG


def _in_maps(stack_input, W1, b1, W2, b2, fm_cat_w0, fm_cat_w, fm_cat_V,
             fm_dm_w0, fm_dm_w, fm_dm_V):
    f = lambda a: np.ascontiguousarray(np.asarray(a, np.float32))
    stack_input = f(stack_input)
    wpack, statp, comb2, biasp, w0col = _host_prep(
        f(W1), f(b1), f(W2), f(b2), f(fm_cat_w0), f(fm_cat_w), f(fm_cat_V),
        f(fm_dm_w0), f(fm_dm_w), f(fm_dm_V))
    common = {"wpack": wpack, "statp": statp, "comb2": comb2, "biasp": biasp,
              "w0col": w0col}
    return [dict(common, x=np.ascontiguousarray(stack_input[:, b]))
            for b in range(N_CORES)]


def kernel(stack_input, W1, b1, W2, b2, fm_cat_w0, fm_cat_w, fm_cat_V,
           fm_dm_w0, fm_dm_w, fm_dm_V):
    from concourse.bass_utils import run_bass_kernel_spmd

    in_maps = _in_maps(stack_input, W1, b1, W2, b2, fm_cat_w0, fm_cat_w,
                       fm_cat_V, fm_dm_w0, fm_dm_w, fm_dm_V)
    nc = _get_prog()
    res = run_bass_kernel_spmd(nc, in_maps, core_ids=list(range(N_CORES)))
    outs = [r["out"] for r in res.results]            # each [2, 3, L]
    fp = np.stack([o[0].T for o in outs]).astype(np.float32)   # [8, L, 3]
    fq = np.stack([o[1].T for o in outs]).astype(np.float32)
    return fp, fq


# revision 28
# speedup vs baseline: 1.4664x; 1.4040x over previous
"""Trainium2 Bass kernel for nn_BAC_15152644620305.

Per batch element (1 per NeuronCore, 8 cores):
  p_dense = relu(p @ W1 + b1); q_dense = relu(q @ W2 + b2)
  A = (p_dense @ q_dense.T) / sqrt(600)
  passage_aligned = softmax_rows(A) @ passage ; query_aligned = softmax_cols(A).T @ query
  6 factorization-machine heads on {concat, diff, mul} pairs -> [L, 3] x 2 outputs.

Implementation notes:
  - All heavy matmuls in bf16 (1 cyc/row on PE), fp32 PSUM accumulation.
  - Affinity computed in BOTH layouts (cheaper than transposing exp(A) on-chip);
    exp without max-subtraction (affinity values are in [0.1, 1.1]).
  - exp(A) stored as fp8e4m3 (softmax weights only -> negligible error, halves SBUF).
  - Softmax denominators ride along as an extra ones-column in the aligned matmuls'
    stationary operand, landing at an aligned output partition (96).
  - FM heads algebraically reduced: the x^2 @ V^2.T term needs only sum_k V_k^2;
    diff projections are linear combos of the qa/p projections; all per-head
    combination is done by one small stationary matmul per output chunk.
"""
import numpy as np

L_FULL = 2048
D = 600
U = 300
KFM = 5
N_CORES = 8
SCALE = float(1.0 / np.sqrt(np.float32(D)))

DCH = [(0, 128), (128, 128), (256, 128), (384, 128), (512, 88)]   # D chunks
UCH = [(0, 128), (128, 128), (256, 44)]                           # U chunks
ONES_COL = 608        # column in the 640-wide natural tile holding the ones
ONES_ROW = 96         # output partition where the denominator row lands
NATW = 640


def _emit(nc, L):
    import concourse.bass as bass
    import concourse.mybir as mybir
    import concourse.tile as tile
    from concourse.masks import make_identity
    from contextlib import ExitStack

    f32 = mybir.dt.float32
    bf16 = mybir.dt.bfloat16
    fp8 = mybir.dt.float8e4
    AF = mybir.ActivationFunctionType
    ds = bass.ds

    LT = L // 128               # l tiles
    NCW = min(512, L)           # moving-dim chunk width
    NCX = L // NCW              # chunks per L
    TG = 4 if LT % 4 == 0 else 1  # l-tiles per transpose psum batch

    x_d = nc.dram_tensor("x", [2, L, D], f32, kind="ExternalInput")
    wp_d = nc.dram_tensor("wpack", [10, 128, U], f32, kind="ExternalInput")
    sp_d = nc.dram_tensor("statp", [10, 128, 36], f32, kind="ExternalInput")
    c2_d = nc.dram_tensor("comb2", [128, 6], f32, kind="ExternalInput")
    bp_d = nc.dram_tensor("biasp", [128, 6], f32, kind="ExternalInput")
    w0_d = nc.dram_tensor("w0col", [3, 2], f32, kind="ExternalInput")
    out_d = nc.dram_tensor("out", [2, 3, L], f32, kind="ExternalOutput")

    with tile.TileContext(nc) as tc, ExitStack() as ctx:
        const = ctx.enter_context(tc.tile_pool(name="const", bufs=1))
        big = ctx.enter_context(tc.tile_pool(name="big", bufs=1))
        epool = ctx.enter_context(tc.tile_pool(name="epool", bufs=LT))
        natp = ctx.enter_context(tc.tile_pool(name="natp", bufs=LT))
        nf32p = ctx.enter_context(tc.tile_pool(name="nf32p", bufs=3))
        stg = ctx.enter_context(tc.tile_pool(name="stg", bufs=2))
        fmt = ctx.enter_context(tc.tile_pool(name="fmt", bufs=6))
        sp = ctx.enter_context(tc.tile_pool(name="sp", bufs=2))
        rp = ctx.enter_context(tc.tile_pool(name="rp", bufs=2))
        ob = ctx.enter_context(tc.tile_pool(name="ob", bufs=1))
        ps = ctx.enter_context(tc.tile_pool(name="ps", bufs=8, space="PSUM"))

        def pst(p_cnt=128, w=NCW):
            return ps.tile([p_cnt, w], f32, tag="ps", name="pst")

        # ------- constants (packed loads on the scalar HWDGE queue) -------
        identb = const.tile([128, 128], bf16, tag="identb")
        make_identity(nc, identb)
        onesb = const.tile([128, 128], bf16, tag="onesb")
        nc.vector.memset(onesb[:], 1.0)
        w0sb = const.tile([3, 2], f32, tag="w0sb")
        nc.scalar.dma_start(w0sb[:], w0_d[:])

        wstg = stg.tile([128, 10 * U], f32, tag="stg_w", name="wstg", bufs=1)
        nc.scalar.dma_start(
            wstg[:].rearrange("p (t c) -> p t c", t=10),
            wp_d[:].rearrange("t p c -> p t c"))
        Wall = const.tile([128, 10 * U], bf16, tag="Wall")
        nc.vector.tensor_copy(Wall[:], wstg[:])
        Wsb = [[Wall[:, ds((t * 5 + k) * U, U)] for k in range(5)]
               for t in range(2)]

        sstg = stg.tile([128, 360], f32, tag="stg_s", name="sstg", bufs=1)
        nc.scalar.dma_start(
            sstg[:].rearrange("p (t c) -> p t c", t=10),
            sp_d[:].rearrange("t p c -> p t c"))
        Sall = const.tile([128, 360], bf16, tag="Sall")
        nc.vector.tensor_copy(Sall[:], sstg[:])
        stat = [[Sall[:, ds((s * 5 + k) * 36, 36)] for k in range(5)]
                for s in range(2)]

        cstg = stg.tile([128, 6], f32, tag="stg_c", name="cstg", bufs=1)
        nc.scalar.dma_start(cstg[:], c2_d[:])
        cb2 = const.tile([128, 6], bf16, tag="cb2")
        nc.vector.tensor_copy(cb2[:], cstg[:])

        bsb = const.tile([128, 6], f32, tag="bsb")
        nc.scalar.dma_start(bsb[:], bp_d[:])

        # ---------------- phase 1: transpose inputs -> pT/qT (bf16 [d, L]) ----
        xT = [[], []]
        for t in range(2):
            for k in range(len(DCH)):
                xT[t].append(big.tile([128, L], bf16, tag=f"xT{t}_{k}",
                                      name=f"xT{t}_{k}"))
        # phase 1+2 interleaved per l-group: transpose inputs -> pT/qT, then
        # the dense matmuls for that group's columns (keeps PE fed during the
        # next group's DMA + cast)
        dT = [[], []]
        for t in range(2):
            for m in range(len(UCH)):
                dT[t].append(big.tile([128, L], bf16, tag=f"dT{t}_{m}",
                                      name=f"dT{t}_{m}"))
        for g in range(LT // TG):
            gw = TG * 128
            for t in range(2):
                # 2 d-chunks per bf16 psum tile (same 2KB bank footprint as
                # one f32 slot) -> 3 slots instead of 5, more slot headroom
                # for the dense accumulators and the next group's transposes
                pjs2 = [ps.tile([128, 2 * NCW], bf16, tag="ps", name="pjs")
                        for _ in range((len(DCH) + 1) // 2)]
                pjs = [pjs2[k // 2][:, ds((k % 2) * NCW, NCW)]
                       for k in range(len(DCH))]
                for ii in range(TG):
                    i = g * TG + ii
                    nf = nf32p.tile([128, D], f32, tag="nf", name="nf")
                    nc.sync.dma_start(nf[:], x_d[t, ds(i * 128, 128), :])
                    nfb = nf32p.tile([128, D], bf16, tag="nfb", name="nfb")
                    nc.vector.tensor_copy(nfb[:], nf[:])
                    for k, (doff, dcnt) in enumerate(DCH):
                        nc.tensor.transpose(
                            pjs[k][:dcnt, ds(ii * 128, 128)],
                            nfb[:, ds(doff, dcnt)], identb[:])
                for k, (doff, dcnt) in enumerate(DCH):
                    # DVE, not ACT: the scalar engine is the busier one overall
                    # (exp + relu evictions); DVE is idle during this phase
                    nc.vector.tensor_copy(xT[t][k][:dcnt, ds(g * gw, gw)],
                                          pjs[k][:dcnt, ds(0, gw)])
            if gw == NCW:
                for t in range(2):
                    for m, (uoff, ucnt) in enumerate(UCH):
                        acc = pst()
                        for k, (doff, dcnt) in enumerate(DCH):
                            nc.tensor.matmul(
                                acc[:ucnt, :],
                                Wsb[t][k][:dcnt, ds(uoff, ucnt)],
                                xT[t][k][:dcnt, ds(g * NCW, NCW)],
                                start=(k == 0), stop=(k == len(DCH) - 1))
                        nc.scalar.activation(
                            dT[t][m][:ucnt, ds(g * NCW, NCW)], acc[:ucnt, :],
                            AF.Relu, bias=bsb[:ucnt, t * 3 + m: t * 3 + m + 1])
        if TG * 128 != NCW:
            for t in range(2):
                for m, (uoff, ucnt) in enumerate(UCH):
                    for nx in range(NCX):
                        acc = pst()
                        for k, (doff, dcnt) in enumerate(DCH):
                            nc.tensor.matmul(
                                acc[:ucnt, :],
                                Wsb[t][k][:dcnt, ds(uoff, ucnt)],
                                xT[t][k][:dcnt, ds(nx * NCW, NCW)],
                                start=(k == 0), stop=(k == len(DCH) - 1))
                        nc.scalar.activation(
                            dT[t][m][:ucnt, ds(nx * NCW, NCW)], acc[:ucnt, :],
                            AF.Relu, bias=bsb[:ucnt, t * 3 + m: t * 3 + m + 1])

        # helpers ------------------------------------------------------------
        def affinity_to_E(lhs_t, rhs_t, tagged):
            """E[i] tiles [128, L] fp8 = exp(SCALE * lhs.T @ rhs) per l-tile."""
            E = []
            for i in range(LT):
                e = epool.tile([128, L], fp8, tag="E", name=f"E{tagged}_{i}")
                for nx in range(NCX):
                    acc = pst()
                    for m, (uoff, ucnt) in enumerate(UCH):
                        nc.tensor.matmul(
                            acc[:, :],
                            lhs_t[m][:ucnt, ds(i * 128, 128)],
                            rhs_t[m][:ucnt, ds(nx * NCW, NCW)],
                            start=(m == 0), stop=(m == len(UCH) - 1))
                    nc.scalar.activation(e[:, ds(nx * NCW, NCW)], acc[:, :],
                                         AF.Exp, scale=SCALE)
                E.append(e)
            return E

        def build_nat(t):
            """natural-layout bf16 tiles [128, 640] with ones col, from x[t]."""
            nats = []
            for i in range(LT):
                nf = nf32p.tile([128, D], f32, tag="nf", name="nfn")
                nc.sync.dma_start(nf[:], x_d[t, ds(i * 128, 128), :])
                nt = natp.tile([128, NATW], bf16, tag="nat", name=f"nat{t}_{i}")
                nc.vector.memset(nt[:], 0.0)
                nc.vector.tensor_copy(nt[:, 0:D], nf[:])
                nc.vector.memset(nt[:, ONES_COL:ONES_COL + 1], 1.0)
                nats.append(nt)
            return nats

        def aligned_T(nats, E, side_tag):
            """alT tiles [d,L] bf16 = normalized aligned.T, via ones-row trick."""
            alT = [big.tile([128, L], bf16, tag=f"alT{k}", name=f"alT{side_tag}{k}")
                   for k in range(len(DCH))]
            R = big.tile([128, L], bf16, tag="R", name=f"R{side_tag}")
            # pass A: last d-chunk (88 rows) + ones row at partition 96
            ps4 = [pst() for _ in range(NCX)]
            for i in range(LT):
                for nx in range(NCX):
                    nc.tensor.matmul(ps4[nx][:, :],
                                     nats[i][:, ds(512, 128)],
                                     E[i][:, ds(nx * NCW, NCW)],
                                     start=(i == 0), stop=(i == LT - 1))
            for nx in range(NCX):
                rr = rp.tile([128, NCW], f32, tag="rr", name="rr")
                nc.vector.reciprocal(rr[ONES_ROW:ONES_ROW + 1, :],
                                     ps4[nx][ONES_ROW:ONES_ROW + 1, :])
                rrb = rp.tile([128, NCW], bf16, tag="rrb", name="rrb")
                nc.scalar.copy(rrb[ONES_ROW:ONES_ROW + 1, :],
                               rr[ONES_ROW:ONES_ROW + 1, :])
                bc = pst()
                nc.tensor.matmul(bc[:, :], onesb[ONES_ROW:ONES_ROW + 1, 0:128],
                                 rrb[ONES_ROW:ONES_ROW + 1, :],
                                 start=True, stop=True,
                                 tile_position=(ONES_ROW, 0))
                nc.scalar.copy(R[:, ds(nx * NCW, NCW)], bc[:, :])
                nc.vector.tensor_mul(alT[4][0:88, ds(nx * NCW, NCW)],
                                     ps4[nx][0:88, :], R[0:88, ds(nx * NCW, NCW)])
            # passes B, C: d-chunks 0..3, two at a time
            for mm0 in (0, 2):
                accs = {}
                for m in (mm0, mm0 + 1):
                    for nx in range(NCX):
                        accs[(m, nx)] = pst()
                for i in range(LT):
                    for m in (mm0, mm0 + 1):
                        for nx in range(NCX):
                            nc.tensor.matmul(accs[(m, nx)][:, :],
                                             nats[i][:, ds(m * 128, 128)],
                                             E[i][:, ds(nx * NCW, NCW)],
                                             start=(i == 0), stop=(i == LT - 1))
                for m in (mm0, mm0 + 1):
                    for nx in range(NCX):
                        nc.vector.tensor_mul(alT[m][:, ds(nx * NCW, NCW)],
                                             accs[(m, nx)][:, :],
                                             R[:, ds(nx * NCW, NCW)])
            return alT

        def fm_side(s, xTs, bTs):
            """FM heads for one side: x = aligned.T tiles, b = own input.T."""
            for nx in range(NCX):
                nsl = ds(nx * NCW, NCW)
                # col-packed projection groups: two psum tiles, three
                # concurrent col-groups each (col 96 / quadrant 3 avoided)
                # P1: X@0 (12), B@32 (12), M2@64 (1)
                # P2: X2@0 (2), B2@32 (2), M@64 (7)
                P1 = ps.tile([128, NCW], f32, tag="ps", name="P1")
                P2 = ps.tile([128, NCW], f32, tag="ps", name="P2")
                nk = len(DCH)
                for k, (doff, dcnt) in enumerate(DCH):
                    x_sl = xTs[k][:dcnt, nsl]
                    b_sl = bTs[k][:dcnt, nsl]
                    tx2 = fmt.tile([128, NCW], bf16, tag="fmt", name="tx2")
                    tb2 = fmt.tile([128, NCW], bf16, tag="fmt", name="tb2")
                    txm = fmt.tile([128, NCW], bf16, tag="fmt", name="txm")
                    txm2 = fmt.tile([128, NCW], bf16, tag="fmt", name="txm2")
                    nc.vector.tensor_mul(tx2[:dcnt, :], x_sl, x_sl)
                    nc.vector.tensor_mul(tb2[:dcnt, :], b_sl, b_sl)
                    nc.vector.tensor_mul(txm[:dcnt, :], x_sl, b_sl)
                    nc.vector.tensor_mul(txm2[:dcnt, :], txm[:dcnt, :],
                                         txm[:dcnt, :])
                    st = stat[s][k]
                    fl = (k == 0, k == nk - 1)
                    nc.tensor.matmul(P1[0:12, :], st[:dcnt, 0:12], x_sl,
                                     start=fl[0], stop=fl[1],
                                     tile_position=(0, 0),
                                     skip_group_check=True)
                    nc.tensor.matmul(P1[32:44, :], st[:dcnt, 12:24], b_sl,
                                     start=fl[0], stop=fl[1],
                                     tile_position=(0, 32),
                                     skip_group_check=True)
                    nc.tensor.matmul(P1[64:65, :], st[:dcnt, 35:36],
                                     txm2[:dcnt, :], start=fl[0], stop=fl[1],
                                     tile_position=(0, 64),
                                     skip_group_check=True)
                    nc.tensor.matmul(P2[0:2, :], st[:dcnt, 24:26],
                                     tx2[:dcnt, :], start=fl[0], stop=fl[1],
                                     tile_position=(0, 0),
                                     skip_group_check=True)
                    nc.tensor.matmul(P2[32:34, :], st[:dcnt, 26:28],
                                     tb2[:dcnt, :], start=fl[0], stop=fl[1],
                                     tile_position=(0, 32),
                                     skip_group_check=True)
                    nc.tensor.matmul(P2[64:71, :], st[:dcnt, 28:35],
                                     txm[:dcnt, :], start=fl[0], stop=fl[1],
                                     tile_position=(0, 64),
                                     skip_group_check=True)
                # pack group evictions at 32-aligned partition offsets so the
                # whole combine is 2 matmuls: S1 = [X@0, B@32, X2@64, B2@96],
                # S2 = [M@0, M2@32, TQ@64, TQM@96]
                S1 = sp.tile([128, NCW], bf16, tag="S1", name="S1")
                S2 = sp.tile([128, NCW], bf16, tag="S2", name="S2")
                nc.vector.memset(S1[:], 0.0)
                nc.vector.memset(S2[:], 0.0)
                # split evictions ACT/DVE so the S-build runs in parallel
                nc.scalar.copy(S1[0:12, :], P1[0:12, :])
                nc.scalar.copy(S1[32:44, :], P1[32:44, :])
                nc.vector.tensor_copy(S1[64:66, :], P2[0:2, :])
                nc.vector.tensor_copy(S1[96:98, :], P2[32:34, :])
                nc.vector.tensor_copy(S2[0:7, :], P2[64:71, :])
                nc.vector.tensor_copy(S2[32:33, :], P1[64:65, :])
                # B-group Vd columns carry -Vd, so diff quads are also an add.
                # in0 from PSUM: two SBUF inputs must share a base partition.
                TA = sp.tile([10, NCW], f32, tag="TA", name="TA")
                nc.vector.tensor_add(TA[0:10, :], P1[0:10, :], S1[32:42, :])
                nc.scalar.activation(S2[64:74, :], TA[:, :], AF.Square)
                nc.scalar.activation(S2[96:101, :], S2[0:5, :], AF.Square)
                cps = ps.tile([3, NCW], f32, tag="ps", name="cps")
                nc.tensor.matmul(cps[:, :], cb2[0:98, 0:3], S1[0:98, :],
                                 start=True, stop=False)
                nc.tensor.matmul(cps[:, :], cb2[0:101, 3:6], S2[0:101, :],
                                 start=False, stop=True)
                o = ob.tile([3, NCW], f32, tag="ob", name="o")
                nc.scalar.activation(o[:, :], cps[:, :], AF.Identity,
                                     bias=w0sb[:, s:s + 1])
                nc.sync.dma_start(out_d[s, :, nsl], o[:, :])

        # ---------------- main flow ----------------
        E1 = affinity_to_E(dT[0], dT[1], "1")     # E1[p-tile][p, q]
        q_nats = build_nat(1)
        qaT = aligned_T(q_nats, E1, "q")          # query_aligned.T
        fm_side(0, qaT, xT[0])                    # passage-side features
        E2 = affinity_to_E(dT[1], dT[0], "2")     # E2[q-tile][q, p]
        p_nats = build_nat(0)
        paT = aligned_T(p_nats, E2, "p")          # passage_aligned.T
        fm_side(1, paT, xT[1])                    # query-side features


def _host_prep(W1, b1, W2, b2, cat_w0, cat_w, cat_V, dm_w0, dm_w, dm_V):
    stat = np.zeros((2, D, 36), np.float32)
    for s in range(2):
        ci, di, mi = s, s, s + 2
        Va = cat_V[ci][:, :D]
        Vb = cat_V[ci][:, D:]
        Vd = dm_V[di]
        Vm = dm_V[mi]
        stat[s, :, 0:5] = Va.T
        stat[s, :, 5:10] = Vd.T
        stat[s, :, 10] = cat_w[ci, :D]
        stat[s, :, 11] = dm_w[di]
        stat[s, :, 12:17] = Vb.T
        stat[s, :, 17:22] = -Vd.T   # negated: quad build is then a single add
        stat[s, :, 22] = cat_w[ci, D:]
        stat[s, :, 23] = dm_w[di]
        stat[s, :, 24] = (Va ** 2).sum(0)
        stat[s, :, 25] = (Vd ** 2).sum(0)
        stat[s, :, 26] = (Vb ** 2).sum(0)
        stat[s, :, 27] = (Vd ** 2).sum(0)
        stat[s, :, 28:33] = Vm.T
        stat[s, :, 33] = dm_w[mi]
        stat[s, :, 34] = (Vd ** 2).sum(0)
        stat[s, :, 35] = (Vm ** 2).sum(0)

    # packed combine matrices: S1 = [X@0, B@32, X2@64, B2@96],
    # S2 = [M@0, M2@32, TQ@64, TQM@96]
    comb2 = np.zeros((128, 6), np.float32)
    C1, C2 = comb2[:, 0:3], comb2[:, 3:6]
    C1[10, 0] = 1.0     # x@w_cat -> c_cat
    C1[11, 1] = 1.0     # x@w_d -> c_diff
    C1[32 + 10, 0] = 1.0
    C1[32 + 11, 1] = -1.0
    C1[64, 0] = -0.5    # x2@u_cat
    C1[65, 1] = -0.5    # x2@u_d
    C1[96, 0] = -0.5    # b2@u_cat
    C1[97, 1] = -0.5    # b2@u_d
    C2[5, 2] = 1.0      # mul@w_m
    C2[6, 1] = 1.0      # mul@u_d (from -0.5 * -2)
    C2[32, 2] = -0.5    # mul2@u_m
    C2[64:69, 0] = 0.5  # cat quads
    C2[69:74, 1] = 0.5  # diff quads
    C2[96:101, 2] = 0.5  # mul quads

    # packed per-d-chunk weights / stationaries / bias
    wpack = np.zeros((10, 128, U), np.float32)
    statp = np.zeros((10, 128, 36), np.float32)
    for t, W in enumerate((W1, W2)):
        for k, (doff, dcnt) in enumerate(DCH):
            wpack[t * 5 + k, :dcnt] = W[doff:doff + dcnt]
    for s in range(2):
        for k, (doff, dcnt) in enumerate(DCH):
            statp[s * 5 + k, :dcnt] = stat[s, doff:doff + dcnt]

    biasp = np.zeros((128, 6), np.float32)
    for t, b in enumerate((b1, b2)):
        for m, (uoff, ucnt) in enumerate(UCH):
            biasp[:ucnt, t * 3 + m] = b[uoff:uoff + ucnt]

    w0col = np.zeros((3, 2), np.float32)
    for s in range(2):
        w0col[0, s] = cat_w0[s, 0]
        w0col[1, s] = dm_w0[s, 0]
        w0col[2, s] = dm_w0[s + 2, 0]
    return wpack, statp, comb2, biasp, w0col


_PROG = None


def _get_prog():
    global _PROG
    if _PROG is None:
        from concourse import bacc
        nc = bacc.Bacc(None, target_bir_lowering=False)
        _emit(nc, L_FULL)
        nc.finalize()
        _PROG = nc
    return _PROG


def _in_maps(stack_input, W1, b1, W2, b2, fm_cat_w0, fm_cat_w, fm_cat_V,
             fm_dm_w0, fm_dm_w, fm_dm_V):
    f = lambda a: np.ascontiguousarray(np.asarray(a, np.float32))
    stack_input = f(stack_input)
    wpack, statp, comb2, biasp, w0col = _host_prep(
        f(W1), f(b1), f(W2), f(b2), f(fm_cat_w0), f(fm_cat_w), f(fm_cat_V),
        f(fm_dm_w0), f(fm_dm_w), f(fm_dm_V))
    common = {"wpack": wpack, "statp": statp, "comb2": comb2, "biasp": biasp,
              "w0col": w0col}
    return [dict(common, x=np.ascontiguousarray(stack_input[:, b]))
            for b in range(N_CORES)]


def kernel(stack_input, W1, b1, W2, b2, fm_cat_w0, fm_cat_w, fm_cat_V,
           fm_dm_w0, fm_dm_w, fm_dm_V):
    from concourse.bass_utils import run_bass_kernel_spmd

    in_maps = _in_maps(stack_input, W1, b1, W2, b2, fm_cat_w0, fm_cat_w,
                       fm_cat_V, fm_dm_w0, fm_dm_w, fm_dm_V)
    nc = _get_prog()
    res = run_bass_kernel_spmd(nc, in_maps, core_ids=list(range(N_CORES)))
    outs = [r["out"] for r in res.results]            # each [2, 3, L]
    fp = np.stack([o[0].T for o in outs]).astype(np.float32)   # [8, L, 3]
    fq = np.stack([o[1].T for o in outs]).astype(np.float32)
    return fp, fq
